# revision 1
# baseline (speedup 1.0000x reference)
"""Trainium2 Bass kernel for nn_BOW: emb = relu(relu(relu(bow(idx) @ W1.T + b1) @ W2.T + b2) @ W3.T + b3).

Strategy: data-parallel over batch across 8 NeuronCores (32 rows each).
The bow-histogram @ W1.T product is reformulated as an embedding-bag:
    h1[b] = b1 + sum_j W1T[idx[b, j]]
so no histogram scatter is ever materialized.  Each core indirect-DMA-gathers
its 32*512 = 16384 token rows (4 KB each) from a host-prepped table
W1T [V, 2048] bf16 laid out as [hi | lo] where hi = bf16(W1T_f32) and
lo = bf16(W1T_f32 - hi): summing hi- and lo-products in fp32 PSUM recovers
fp32-level accuracy (~3e-6 scale-relative) at bf16 matmul speed (1 cyc/row).
Each gather tile [128 tokens, 2048] interleaves 4 tokens from each of the 32
rows; a static one-hot "membership" lhsT [128, 32] on the TensorEngine reduces
tokens into their rows, accumulating all 128 gathers in PSUM.  fc2/fc3 are
small dense matmuls done per-core on its 32 rows.  No collectives.

Measured: ~205 us/core steady state (8 cores), vs a ~187 us HBM roofline for
the 67 MB/core of gather traffic.  PE ~125 us, SWDGE descriptor gen ~134 us,
both hidden under DMA.
"""
import sys

try:
    import concourse.bass  # noqa: F401  (already importable in some setups)
except ImportError:
    sys.path.insert(0, "/opt/trn_rl_repo")

import numpy as np
import concourse.bass as bass
import concourse.tile as tile
import concourse.mybir as mybir
from concourse import bacc
from concourse.bass_utils import run_bass_kernel_spmd
from concourse.masks import make_identity

N_CORES = 8
B, S = 256, 512
V = 50000
M1, M2, EMB = 1024, 512, 256
RPC = B // N_CORES          # rows per core = 32
TPR = 128 // RPC            # tokens per row per gather = 4
NG = S // TPR               # gathers per core = 128

_CACHE = {}

# fc1 table precision: 2 = bf16 hi+lo split (error ~1e-6, full 4KB/token
# traffic), 1 = bf16 hi only (error ~2e-3, half traffic).
SPLIT = 2


def _build(reps=1):
    nc = bacc.Bacc("TRN2", target_bir_lowering=False, debug=False, num_devices=N_CORES)
    f32 = mybir.dt.float32

    w1t = nc.dram_tensor("w1t", [V, SPLIT * M1], mybir.dt.bfloat16, kind="ExternalInput")
    w2t = nc.dram_tensor("w2t", [M1, M2], f32, kind="ExternalInput")
    w3t = nc.dram_tensor("w3t", [M2, EMB], f32, kind="ExternalInput")
    b1 = nc.dram_tensor("b1", [1, M1], f32, kind="ExternalInput")
    b2 = nc.dram_tensor("b2", [1, M2], f32, kind="ExternalInput")
    b3 = nc.dram_tensor("b3", [1, EMB], f32, kind="ExternalInput")
    gidx = nc.dram_tensor("gidx", [128, NG], mybir.dt.int32, kind="ExternalInput")
    memb = nc.dram_tensor("memb", [128, RPC], mybir.dt.bfloat16, kind="ExternalInput")
    emb = nc.dram_tensor("emb", [reps * RPC, EMB], f32, kind="ExternalOutput")

    with tile.TileContext(nc) as tc:
        with (
            tc.tile_pool(name="const", bufs=1) as cpool,
            tc.tile_pool(name="gath", bufs=8) as gpool,
            tc.tile_pool(name="act", bufs=1) as apool,
            tc.tile_pool(name="ph1", bufs=1, space="PSUM") as ph1_pool,
            tc.tile_pool(name="ptr", bufs=2, space="PSUM") as ptr_pool,
            tc.tile_pool(name="psm", bufs=1, space="PSUM") as psm_pool,
        ):
            # ---- constants ----
            idx_t = cpool.tile([128, NG], mybir.dt.int32)
            nc.sync.dma_start(idx_t[:], gidx[:])
            memb_t = cpool.tile([128, RPC], mybir.dt.bfloat16)
            nc.sync.dma_start(memb_t[:], memb[:])
            b1_t = cpool.tile([1, M1], f32)
            nc.sync.dma_start(b1_t[:], b1[:])
            b2_t = cpool.tile([1, M2], f32)
            nc.sync.dma_start(b2_t[:], b2[:])
            b3_t = cpool.tile([1, EMB], f32)
            nc.sync.dma_start(b3_t[:], b3[:])
            w2_t = cpool.tile([128, M1 // 128, M2], f32)
            for a in range(M1 // 128):
                nc.sync.dma_start(w2_t[:, a, :], w2t[a * 128:(a + 1) * 128, :])
            w3_t = cpool.tile([128, M2 // 128, EMB], f32)
            for a in range(M2 // 128):
                nc.sync.dma_start(w3_t[:, a, :], w3t[a * 128:(a + 1) * 128, :])
            ones1 = cpool.tile([1, RPC], f32)
            nc.gpsimd.memset(ones1[:], 1.0)
            ident = cpool.tile([RPC, RPC], f32)
            make_identity(nc, ident[:])

            for _rep in range(reps):
                _body(nc, tc, gpool, apool, ph1_pool, ptr_pool, psm_pool,
                      idx_t, memb_t, b1_t, b2_t, b3_t, w2_t, w3_t, ones1, ident,
                      emb[_rep * RPC:(_rep + 1) * RPC, :], w1t)

    nc.compile()
    return nc


def _body(nc, tc, gpool, apool, ph1_pool, ptr_pool, psm_pool,
          idx_t, memb_t, b1_t, b2_t, b3_t, w2_t, w3_t, ones1, ident, emb, w1t):
    f32 = mybir.dt.float32
    # ---- fc1: gather + membership matmul, accumulate in PSUM ----
    ph1a = ph1_pool.tile([RPC, 512], f32, tag="h1a")
    ph1b = ph1_pool.tile([RPC, 512], f32, tag="h1b")
    ph1 = [ph1a, ph1b]
    for k in range(NG):
        gt = gpool.tile([128, SPLIT * M1], mybir.dt.bfloat16)
        nc.gpsimd.indirect_dma_start(
            out=gt[:], out_offset=None, in_=w1t[:],
            in_offset=bass.IndirectOffsetOnAxis(ap=idx_t[:, k:k + 1], axis=0),
        )
        for s in range(SPLIT):
            for h in range(2):
                nc.tensor.matmul(
                    ph1[h][:],
                    lhsT=memb_t[:],
                    rhs=gt[:, (2 * s + h) * 512:(2 * s + h + 1) * 512],
                    start=(k == 0 and s == 0), stop=False,
                )
    h1 = apool.tile([RPC, M1], f32)
    for h in range(2):
        nc.tensor.matmul(
            ph1[h][:], lhsT=ones1[:], rhs=b1_t[:, h * 512:(h + 1) * 512],
            start=False, stop=True,
        )
        nc.scalar.activation(
            h1[:, h * 512:(h + 1) * 512], ph1[h][:],
            mybir.ActivationFunctionType.Relu,
        )

    # ---- transpose h1 -> h1t [128, 8, RPC] ----
    h1t = apool.tile([128, M1 // 128, RPC], f32)
    for a in range(M1 // 128):
        pt = ptr_pool.tile([128, RPC], f32, tag="tr")
        nc.tensor.transpose(pt[:], h1[:, a * 128:(a + 1) * 128], ident[:])
        nc.vector.tensor_copy(h1t[:, a, :], pt[:])

    # ---- fc2 ----
    ph2 = psm_pool.tile([RPC, M2], f32, tag="h2")
    for a in range(M1 // 128):
        nc.tensor.matmul(
            ph2[:], lhsT=h1t[:, a, :], rhs=w2_t[:, a, :],
            start=(a == 0), stop=False,
        )
    nc.tensor.matmul(ph2[:], lhsT=ones1[:], rhs=b2_t[:], start=False, stop=True)
    h2 = apool.tile([RPC, M2], f32)
    nc.scalar.activation(h2[:], ph2[:], mybir.ActivationFunctionType.Relu)

    # ---- transpose h2 -> h2t [128, 4, RPC] ----
    h2t = apool.tile([128, M2 // 128, RPC], f32)
    for a in range(M2 // 128):
        pt = ptr_pool.tile([128, RPC], f32, tag="tr")
        nc.tensor.transpose(pt[:], h2[:, a * 128:(a + 1) * 128], ident[:])
        nc.vector.tensor_copy(h2t[:, a, :], pt[:])

    # ---- fc3 ----
    ph3 = psm_pool.tile([RPC, EMB], f32, tag="h3")
    for a in range(M2 // 128):
        nc.tensor.matmul(
            ph3[:], lhsT=h2t[:, a, :], rhs=w3_t[:, a, :],
            start=(a == 0), stop=False,
        )
    nc.tensor.matmul(ph3[:], lhsT=ones1[:], rhs=b3_t[:], start=False, stop=True)
    out_t = apool.tile([RPC, EMB], f32)
    nc.scalar.activation(out_t[:], ph3[:], mybir.ActivationFunctionType.Relu)
    nc.sync.dma_start(emb[:], out_t[:])


def _prep_inputs(idx, W1, b1, W2, b2, W3, b3):
    """Host-side sharding/layout prep. Returns per-core input maps."""
    import ml_dtypes

    idx = np.asarray(idx)
    w1f = np.asarray(W1, dtype=np.float32).T                          # [V, M1]
    hi = w1f.astype(ml_dtypes.bfloat16)
    if SPLIT == 2:
        lo = (w1f - hi.astype(np.float32)).astype(ml_dtypes.bfloat16)
        w1t = np.ascontiguousarray(np.concatenate([hi, lo], axis=1))  # [V, 2*M1]
    else:
        w1t = np.ascontiguousarray(hi)                                # [V, M1]
    w2t = np.ascontiguousarray(np.asarray(W2, dtype=np.float32).T)   # [M1, M2]
    w3t = np.ascontiguousarray(np.asarray(W3, dtype=np.float32).T)   # [M2, EMB]
    b1r = np.asarray(b1, dtype=np.float32).reshape(1, M1)
    b2r = np.asarray(b2, dtype=np.float32).reshape(1, M2)
    b3r = np.asarray(b3, dtype=np.float32).reshape(1, EMB)
    # membership: partition p belongs to row p // TPR
    memb = (np.arange(128)[:, None] // TPR == np.arange(RPC)[None, :]).astype(
        ml_dtypes.bfloat16
    )
    in_maps = []
    for c in range(N_CORES):
        rows = idx[c * RPC:(c + 1) * RPC]                 # [RPC, S]
        # gidx[p, k] = rows[p // TPR, TPR*k + p % TPR]
        g = rows.reshape(RPC, NG, TPR).transpose(0, 2, 1)  # [RPC, TPR, NG]
        gidx = g.reshape(128, NG).astype(np.int32)
        in_maps.append({
            "w1t": w1t, "w2t": w2t, "w3t": w3t,
            "b1": b1r, "b2": b2r, "b3": b3r,
            "gidx": np.ascontiguousarray(gidx), "memb": memb,
        })
    return in_maps


def kernel(idx, W1, b1, W2, b2, W3, b3):
    if "nc" not in _CACHE:
        _CACHE["nc"] = _build()
    nc = _CACHE["nc"]
    in_maps = _prep_inputs(idx, W1, b1, W2, b2, W3, b3)
    try:
        res = run_bass_kernel_spmd(nc, in_maps, list(range(N_CORES)))
    except Exception:
        # one retry: transient device errors (wedged NeuronCore from a prior
        # crashed process) usually clear on re-execution
        res = run_bass_kernel_spmd(nc, in_maps, list(range(N_CORES)))
    return np.concatenate([res.results[c]["emb"] for c in range(N_CORES)], axis=0)

